# revision 15
# baseline (speedup 1.0000x reference)
"""Causal self-attention (B=8, T=1024, D=2048, H=16) on 8 NeuronCores.

Data-parallel over the batch dim: core i handles batch element i end-to-end
(QKV proj -> causal attention -> out proj). No collectives.

Layout: everything runs on transposed activations. The host feeds x[b].T
([D, T]) in bf16; Q/K are produced d-major ([Dh, T]), V token-major, and
the output projection emits y.T which the host transposes back. Every
contraction sits on the partition dim with zero on-device transposes.

Matmuls run in bf16 (same PE rate as fp32r but faster weight loads and
half the DMA/SBUF traffic) with fp32 PSUM accumulation; rel err lands at
~5e-3, within the 2e-2 gate. The bf16 activations are small enough that
the attention output stays resident in SBUF (no DRAM spill between the
attention and the out-proj).

Per-head Q/K/out-proj weight blocks are pre-gathered on the host into a
[p, ctile, f] layout so each block is one DMA with 4KB-contiguous
partition lines (128 descriptors instead of 1024 for the strided view).

Softmax skips the max-subtraction (scores are ~N(0,1); exp is safely in
range). The denominator comes from a ones-column matmul fused into the
same PSUM pass as attn @ V; the 1/denominator broadcast runs on GpSimd.
Diagonal score tiles only compute their valid column range (k <= q), so
the causally-dead triangle costs no PE/scalar/gpsimd work.
"""

import math

import numpy as np

B, T, D = 8, 1024, 2048
H = 16
DH = D // H  # 128
NQ = T // 512  # q columns of 512
NCT = D // 128  # 16 c-tiles
SCALE = 1.0 / math.sqrt(DH)
N_CORES = 8

_CACHE = {}


def _build():
    import concourse.bacc as bacc
    import concourse.mybir as mybir
    import concourse.tile as tile

    f32 = mybir.dt.float32
    bf16 = mybir.dt.bfloat16
    Exp = mybir.ActivationFunctionType.Exp
    from concourse.alu_op_type import AluOpType

    nc = bacc.Bacc(None, target_bir_lowering=False)

    xT = nc.declare_dram_parameter("xT", [D, T], bf16, isOutput=False)
    # w_qkv's V-columns, unchanged layout (row slices are contiguous)
    wv = nc.declare_dram_parameter("wv", [D, D], bf16, isOutput=False)
    # per-head Q/K blocks, host-gathered: [s, h, p, ct*128+f] =
    #   w_qkv[ct*128+p, s*D + h*128 + f]
    wqk = nc.declare_dram_parameter("wqk", [2, H, 128, D], bf16, isOutput=False)
    # out-proj blocks, host-gathered: [dt, p, ct*128+f] = w_proj[ct*128+p, dt*128+f]
    wproj_t = nc.declare_dram_parameter(
        "wproj_t", [D // 128, 128, D], bf16, isOutput=False
    )
    # biases host-prearranged: [128, n] with column j = feature-tile j
    # (f = j*128 + p), so the DMA is 128 contiguous partition lines
    bqkv_pre = nc.declare_dram_parameter(
        "bqkv_pre", [128, 3 * D // 128], f32, isOutput=False
    )
    bproj_pre = nc.declare_dram_parameter(
        "bproj_pre", [128, D // 128], f32, isOutput=False
    )
    bv_pre = nc.declare_dram_parameter("bv_pre", [1, D], f32, isOutput=False)
    outT = nc.declare_dram_parameter("outT", [D, T], f32, isOutput=True)

    with tile.TileContext(nc) as tc:
        with (
            tc.tile_pool(name="xbig", bufs=1) as pool_xbig,
            tc.tile_pool(name="vbig", bufs=1) as pool_vbig,
            tc.tile_pool(name="aobig", bufs=1) as pool_aobig,
            tc.tile_pool(name="qk", bufs=4) as pool_qk,
            tc.tile_pool(name="e", bufs=6) as pool_e,
            tc.tile_pool(name="w512", bufs=2) as pool_w512,
            tc.tile_pool(name="wbig", bufs=2) as pool_wbig,
            tc.tile_pool(name="wproj", bufs=2) as pool_wproj,
            tc.tile_pool(name="outp", bufs=2) as pool_out,
            tc.tile_pool(name="den", bufs=2) as pool_den,
            tc.tile_pool(name="misc", bufs=1) as pool_misc,
        ):
            # ---- constants / biases ----
            # all-ones [128, 128] stationary: the denominator matmul emits
            # its result replicated across all 128 partitions, which IS the
            # broadcast the normalize step needs (no gpsimd broadcast)
            ones_mat_f = pool_misc.tile([128, 128], f32, tag="ones_mat_f")
            nc.vector.memset(ones_mat_f[:], 1.0)
            ones_mat = pool_misc.tile([128, 128], bf16, tag="ones_mat")
            nc.vector.tensor_copy(ones_mat[:], ones_mat_f[:])

            # HAM warmup: a burst of dependency-free matmuls on a zeroed
            # tile gets the PE clock-gate to 8/8 while the first input
            # DMAs are still streaming in.
            warm = pool_misc.tile([128, 128], bf16, tag="warm")
            nc.vector.memset(warm[:], 0.0)
            with tc.tile_pool(name="warmps", bufs=2, space="PSUM") as pool_wm:
                for i in range(32):
                    ps_w = pool_wm.tile([128, 128], f32, tag="warmps")
                    nc.tensor.matmul(
                        ps_w[:], warm[:], warm[:], start=True, stop=True
                    )

            bqkv_sb = pool_misc.tile([128, 3 * D // 128], f32, tag="bqkv")
            nc.sync.dma_start(bqkv_sb[:], bqkv_pre[:])
            bproj_sb = pool_misc.tile([128, D // 128], f32, tag="bproj")
            nc.sync.dma_start(bproj_sb[:], bproj_pre[:])
            # V-bias rows, partition-broadcast to [128, 512] per fc on GpSimd
            bv_row = pool_misc.tile([1, D], f32, tag="bv_row")
            nc.sync.dma_start(bv_row[:], bv_pre[:])
            bv_bcast = pool_misc.tile([128, D], f32, tag="bv_bcast")
            for fc in range(D // 512):
                nc.gpsimd.partition_broadcast(
                    bv_bcast[:, fc * 512 : (fc + 1) * 512],
                    bv_row[:, fc * 512 : (fc + 1) * 512],
                )

            # ---- load x.T resident: 16 tiles [128, 1024], one per c-tile.
            # fc=0 weight DMAs are interleaved ahead of each xT tile and xT
            # is loaded in halves so the first matmuls start early. ----
            xT_t = []
            w_fc0 = []
            for ct in range(NCT):
                w_t = pool_w512.tile(
                    [128, 512], bf16, name="w_fc0", tag="w512", bufs=24
                )
                nc.sync.dma_start(
                    w_t[:], wv[ct * 128 : (ct + 1) * 128, 0:512]
                )
                w_fc0.append(w_t)
                t_ = pool_xbig.tile(
                    [128, T], bf16, name="xT_t", tag="xbig", bufs=NCT
                )
                for half in range(2):
                    nc.sync.dma_start(
                        t_[:, half * 512 : (half + 1) * 512],
                        xT[
                            ct * 128 : (ct + 1) * 128,
                            half * 512 : (half + 1) * 512,
                        ],
                    )
                xT_t.append(t_)

            # ---- phase 1: V for all heads, token-major [128, 8, 2048] ----
            V_sb = pool_vbig.tile([128, T // 128, D], bf16, tag="vbig")
            with tc.tile_pool(name="p1psum", bufs=8, space="PSUM") as pool_p1:
                # fc = 0: ct-outer so compute starts as soon as each xT
                # tile lands (only xT_t[ct] + w_fc0[ct] gate the chain)
                ps_v = [
                    pool_p1.tile([128, 512], f32, name="vps", tag="vps")
                    for _ in range(T // 128)
                ]
                for ct in range(NCT - 1):
                    for tt in range(T // 128):
                        nc.tensor.matmul(
                            ps_v[tt][:],
                            xT_t[ct][:, tt * 128 : (tt + 1) * 128],
                            w_fc0[ct][:],
                            start=(ct == 0),
                            stop=False,
                        )
                for tt in range(T // 128):
                    # final ct closes bank tt, then its evacuation (DVE)
                    # overlaps the next bank's closing matmul
                    nc.tensor.matmul(
                        ps_v[tt][:],
                        xT_t[NCT - 1][:, tt * 128 : (tt + 1) * 128],
                        w_fc0[NCT - 1][:],
                        start=False,
                        stop=True,
                    )
                    # V := psum + b_v (bias broadcast prestaged on gpsimd)
                    nc.vector.tensor_tensor(
                        V_sb[:, tt, 0:512],
                        ps_v[tt][:],
                        bv_bcast[:, 0:512],
                        AluOpType.add,
                    )
                # fc = 1..3: tt-outer so each PSUM bank's chain closes
                # early and its DVE evacuation overlaps the next chain
                for fc in range(1, D // 512):
                    w_list = []
                    for ct in range(NCT):
                        w_t = pool_w512.tile(
                            [128, 512], bf16, name="w_t", tag="w512", bufs=24
                        )
                        nc.sync.dma_start(
                            w_t[:],
                            wv[
                                ct * 128 : (ct + 1) * 128,
                                fc * 512 : (fc + 1) * 512,
                            ],
                        )
                        w_list.append(w_t)
                    for tt in range(T // 128):
                        ps = pool_p1.tile([128, 512], f32, name="vps", tag="vps")
                        for ct in range(NCT):
                            nc.tensor.matmul(
                                ps[:],
                                xT_t[ct][:, tt * 128 : (tt + 1) * 128],
                                w_list[ct][:],
                                start=(ct == 0),
                                stop=(ct == NCT - 1),
                            )
                        nc.vector.tensor_tensor(
                            V_sb[:, tt, fc * 512 : (fc + 1) * 512],
                            ps[:],
                            bv_bcast[:, fc * 512 : (fc + 1) * 512],
                            AluOpType.add,
                        )

            # ---- phase 2: per-head attention; y.T stays resident ----
            ao_full = [
                pool_aobig.tile([128, T], bf16, name="ao_full", tag="aobig", bufs=H)
                for _ in range(H)
            ]
            with (
                tc.tile_pool(name="sps", bufs=3, space="PSUM") as pool_s,
                tc.tile_pool(name="qaps", bufs=2, space="PSUM") as pool_qa,
                tc.tile_pool(name="yps", bufs=2, space="PSUM") as pool_y,
                tc.tile_pool(name="dps", bufs=1, space="PSUM") as pool_d,
            ):
                for h in range(H):
                    # 2a: Q^T and K^T for head h, d-major [128, 1024]
                    qk = {}
                    for s, (si, btile) in (
                        ("q", (0, h)),
                        ("k", (1, NCT + h)),
                    ):
                        sb = pool_qk.tile([128, T], bf16, tag="qk")
                        w_t = pool_wbig.tile(
                            [128, NCT, 128], bf16, name="w_t", tag="wbig", bufs=5
                        )
                        nc.sync.dma_start(
                            w_t[:],
                            wqk[si, h].rearrange("p (n f) -> p n f", n=NCT),
                        )
                        for jc in range(NQ):
                            ps = pool_qa.tile(
                                [128, 512], f32, name="qkps", tag="qa"
                            )
                            for ct in range(NCT):
                                nc.tensor.matmul(
                                    ps[:],
                                    w_t[:, ct, :],
                                    xT_t[ct][:, jc * 512 : (jc + 1) * 512],
                                    start=(ct == 0),
                                    stop=(ct == NCT - 1),
                                )
                            nc.vector.tensor_scalar_add(
                                sb[:, jc * 512 : (jc + 1) * 512],
                                ps[:],
                                bqkv_sb[:, btile : btile + 1],
                            )
                        qk[s] = sb

                    # 2b: causal attention, scores transposed [k, q].
                    # Diagonal tiles (r >= 0) only touch columns [128r:512];
                    # everything left of that is causally dead. The loop is
                    # software-pipelined one tile deep so scores(ki+1) sits
                    # between scores(ki) and AV(ki) in the in-order PE queue,
                    # hiding the exp+mask latency and the matmul drain.
                    for jc in range(NQ):
                        nk = 4 * jc + 4  # k-tiles 0 .. 4*jc+3 (rest masked)
                        ps_y = pool_y.tile([128, 512], f32, tag="y")
                        ps_d = pool_d.tile([128, 512], f32, tag="d")
                        e_pend = [None] * nk

                        def emit_scores(ki, jc=jc):
                            r = ki - 4 * jc
                            off = 128 * r if r > 0 else 0
                            ps_s = pool_s.tile([128, 512], f32, tag="mm512")
                            nc.tensor.matmul(
                                ps_s[:, off:512],
                                qk["k"][:, ki * 128 : (ki + 1) * 128],
                                qk["q"][:, jc * 512 + off : (jc + 1) * 512],
                                start=True,
                                stop=True,
                            )
                            e_t = pool_e.tile([128, 512], bf16, tag="e")
                            nc.scalar.activation(
                                e_t[:, off:512], ps_s[:, off:512], Exp, scale=SCALE
                            )
                            if r >= 0:
                                # keep where k <= q; in local cols f' = f-off
                                # that is f' >= p (off == 128r cancels)
                                nc.gpsimd.affine_select(
                                    out=e_t[:, off:512],
                                    in_=e_t[:, off:512],
                                    compare_op=mybir.AluOpType.is_ge,
                                    fill=0.0,
                                    base=0,
                                    pattern=[[1, 512 - off]],
                                    channel_multiplier=-1,
                                )
                            return e_t

                        def emit_av(ki, jc=jc, nk=nk, h=h):
                            r = ki - 4 * jc
                            off = 128 * r if r > 0 else 0
                            e_t = e_pend[ki]
                            nc.tensor.matmul(
                                ps_y[:, off:512],
                                V_sb[:, ki, h * 128 : (h + 1) * 128],
                                e_t[:, off:512],
                                start=(ki == 0),
                                stop=(ki == nk - 1),
                            )
                            nc.tensor.matmul(
                                ps_d[:, off:512],
                                ones_mat[:],
                                e_t[:, off:512],
                                start=(ki == 0),
                                stop=(ki == nk - 1),
                            )

                        for ki in range(nk + 1):
                            if ki < nk:
                                e_pend[ki] = emit_scores(ki)
                            if ki >= 1:
                                emit_av(ki - 1)
                        # approx reciprocal (denominators bounded away from 0
                        # by the diagonal exp term); ps_d rows are identical,
                        # so the reciprocal IS already partition-broadcast
                        d_bcast = pool_den.tile(
                            [128, 512], f32, name="d_bcast", tag="bcast"
                        )
                        nc.vector.reciprocal_approx_fast(out=d_bcast[:], in_=ps_d[:])
                        nc.vector.tensor_mul(
                            ao_full[h][:, jc * 512 : (jc + 1) * 512],
                            ps_y[:],
                            d_bcast[:],
                        )

            # ---- phase 3: output projection, emitted transposed ----
            with tc.tile_pool(name="p3psum", bufs=4, space="PSUM") as pool_p3:
                for dt in range(D // 128):
                    wp_t = pool_wproj.tile(
                        [128, NCT, 128], bf16, name="wp_t", tag="wproj", bufs=4
                    )
                    nc.sync.dma_start(
                        wp_t[:],
                        wproj_t[dt].rearrange("p (n f) -> p n f", n=NCT),
                    )
                    for jc in range(NQ):
                        ps = pool_p3.tile([128, 512], f32, tag="mm512")
                        for ct in range(NCT):
                            nc.tensor.matmul(
                                ps[:],
                                wp_t[:, ct, :],
                                ao_full[ct][:, jc * 512 : (jc + 1) * 512],
                                start=(ct == 0),
                                stop=(ct == NCT - 1),
                            )
                        # bias+store in halves so the final store drains in
                        # two overlapped 128KB DMAs instead of one 256KB
                        o_t = pool_out.tile([128, 512], f32, tag="outp")
                        for hf in range(2):
                            sl = slice(hf * 256, (hf + 1) * 256)
                            nc.vector.tensor_scalar_add(
                                o_t[:, sl], ps[:, sl], bproj_sb[:, dt : dt + 1]
                            )
                            nc.sync.dma_start(
                                outT[
                                    dt * 128 : (dt + 1) * 128,
                                    jc * 512 + hf * 256 : jc * 512 + (hf + 1) * 256,
                                ],
                                o_t[:, sl],
                            )

    nc.compile()
    return nc


def _get_nc():
    if "nc" not in _CACHE:
        _CACHE["nc"] = _build()
    return _CACHE["nc"]


def kernel(x, w_qkv, b_qkv, w_proj, b_proj, _trace=False, _trace_kwargs=None):
    import ml_dtypes
    from concourse.bass_utils import run_bass_kernel_spmd

    bf16 = ml_dtypes.bfloat16
    x = np.asarray(x, dtype=np.float32)
    w_qkv = np.asarray(w_qkv, dtype=np.float32)
    b_qkv = np.asarray(b_qkv, dtype=np.float32)
    w_proj = np.asarray(w_proj, dtype=np.float32)
    b_proj = np.asarray(b_proj, dtype=np.float32)

    # biases pre-arranged to [128, n] feature-tile columns (contiguous
    # partition lines -> cheap DMAs)
    bqkv_pre = np.ascontiguousarray(b_qkv.reshape(3 * D // 128, 128).T)
    bproj_pre = np.ascontiguousarray(b_proj.reshape(D // 128, 128).T)
    bv_pre = np.ascontiguousarray(b_qkv[2 * D : 3 * D].reshape(1, D))

    # host-side weight relayouts (one DMA with contiguous 4KB partition
    # lines per Q/K head block and per out-proj block)
    wv = np.ascontiguousarray(w_qkv[:, 2 * D : 3 * D]).astype(bf16)
    # [s, h, p, ct*128+f] = w_qkv[ct*128+p, s*D + h*128 + f]
    wqk_f = (
        w_qkv[:, : 2 * D]
        .reshape(NCT, 128, 2, H, DH)
        .transpose(2, 3, 1, 0, 4)
        .reshape(2, H, 128, D)
    )
    wqk = np.ascontiguousarray(wqk_f).astype(bf16)
    # [dt, p, ct*128+f] = w_proj[ct*128+p, dt*128+f]
    wproj_f = (
        w_proj.reshape(NCT, 128, D // 128, 128)
        .transpose(2, 1, 0, 3)
        .reshape(D // 128, 128, D)
    )
    wproj_t = np.ascontiguousarray(wproj_f).astype(bf16)

    nc = _get_nc()
    in_maps = []
    for i in range(N_CORES):
        in_maps.append(
            {
                "xT": np.ascontiguousarray(x[i].T).astype(bf16),
                "wv": wv,
                "wqk": wqk,
                "wproj_t": wproj_t,
                "bqkv_pre": bqkv_pre,
                "bproj_pre": bproj_pre,
                "bv_pre": bv_pre,
            }
        )
    res = run_bass_kernel_spmd(
        nc,
        in_maps,
        list(range(N_CORES)),
        trace=_trace,
        **(_trace_kwargs or {}),
    )
    y = np.stack(
        [np.ascontiguousarray(res.results[i]["outT"].T) for i in range(N_CORES)]
    )
    if _trace:
        _CACHE["last_result"] = res
    return y


# revision 17
# speedup vs baseline: 1.1686x; 1.1686x over previous
"""Causal self-attention (B=8, T=1024, D=2048, H=16) on 8 NeuronCores.

Data-parallel over the batch dim: core i handles batch element i end-to-end
(QKV proj -> causal attention -> out proj). No collectives.

Layout: everything runs on transposed activations. The host feeds x[b].T
([D, T]) in bf16; Q/K are produced d-major ([Dh, T]), V token-major, and
the output projection emits y.T which the host transposes back. Every
contraction sits on the partition dim with zero on-device transposes.

Matmuls run in bf16 (same PE rate as fp32r but faster weight loads and
half the DMA/SBUF traffic) with fp32 PSUM accumulation; rel err lands at
~5e-3, within the 2e-2 gate. The bf16 activations are small enough that
the attention output stays resident in SBUF (no DRAM spill between the
attention and the out-proj).

Per-head Q/K/out-proj weight blocks are pre-gathered on the host into a
[p, ctile, f] layout so each block is one DMA with 4KB-contiguous
partition lines (128 descriptors instead of 1024 for the strided view).

Softmax skips the max-subtraction (scores are ~N(0,1); exp is safely in
range). The denominator comes from a ones-column matmul fused into the
same PSUM pass as attn @ V; the 1/denominator broadcast runs on GpSimd.
Diagonal score tiles only compute their valid column range (k <= q), so
the causally-dead triangle costs no PE/scalar/gpsimd work.
"""

import math

import numpy as np

B, T, D = 8, 1024, 2048
H = 16
DH = D // H  # 128
NQ = T // 512  # q columns of 512
NCT = D // 128  # 16 c-tiles
SCALE = 1.0 / math.sqrt(DH)
N_CORES = 8

_CACHE = {}


def _build():
    import concourse.bacc as bacc
    import concourse.mybir as mybir
    import concourse.tile as tile

    f32 = mybir.dt.float32
    bf16 = mybir.dt.bfloat16
    Exp = mybir.ActivationFunctionType.Exp
    from concourse.alu_op_type import AluOpType

    nc = bacc.Bacc(None, target_bir_lowering=False)

    xT = nc.declare_dram_parameter("xT", [D, T], bf16, isOutput=False)
    # w_qkv's V-columns, unchanged layout (row slices are contiguous)
    wv = nc.declare_dram_parameter("wv", [D, D], bf16, isOutput=False)
    # per-head Q/K blocks, host-gathered: [s, h, p, ct*128+f] =
    #   w_qkv[ct*128+p, s*D + h*128 + f]
    wqk = nc.declare_dram_parameter("wqk", [2, H, 128, D], bf16, isOutput=False)
    # out-proj blocks, host-gathered: [dt, p, ct*128+f] = w_proj[ct*128+p, dt*128+f]
    wproj_t = nc.declare_dram_parameter(
        "wproj_t", [D // 128, 128, D], bf16, isOutput=False
    )
    # biases host-prearranged: [128, n] with column j = feature-tile j
    # (f = j*128 + p), so the DMA is 128 contiguous partition lines
    bqkv_pre = nc.declare_dram_parameter(
        "bqkv_pre", [128, 3 * D // 128], f32, isOutput=False
    )
    bproj_pre = nc.declare_dram_parameter(
        "bproj_pre", [128, D // 128], f32, isOutput=False
    )
    bv_pre = nc.declare_dram_parameter("bv_pre", [1, D], f32, isOutput=False)
    outT = nc.declare_dram_parameter("outT", [D, T], f32, isOutput=True)

    with tile.TileContext(nc) as tc:
        with (
            tc.tile_pool(name="xbig", bufs=1) as pool_xbig,
            tc.tile_pool(name="vbig", bufs=1) as pool_vbig,
            tc.tile_pool(name="aobig", bufs=1) as pool_aobig,
            tc.tile_pool(name="qk", bufs=4) as pool_qk,
            tc.tile_pool(name="e", bufs=6) as pool_e,
            tc.tile_pool(name="w512", bufs=2) as pool_w512,
            tc.tile_pool(name="wbig", bufs=2) as pool_wbig,
            tc.tile_pool(name="wproj", bufs=2) as pool_wproj,
            tc.tile_pool(name="outp", bufs=2) as pool_out,
            tc.tile_pool(name="den", bufs=2) as pool_den,
            tc.tile_pool(name="misc", bufs=1) as pool_misc,
        ):
            # ---- constants / biases ----
            # all-ones [128, 128] stationary: the denominator matmul emits
            # its result replicated across all 128 partitions, which IS the
            # broadcast the normalize step needs (no gpsimd broadcast)
            ones_mat_f = pool_misc.tile([128, 128], f32, tag="ones_mat_f")
            nc.vector.memset(ones_mat_f[:], 1.0)
            ones_mat = pool_misc.tile([128, 128], bf16, tag="ones_mat")
            nc.vector.tensor_copy(ones_mat[:], ones_mat_f[:])

            # HAM warmup: a burst of dependency-free matmuls on a zeroed
            # tile gets the PE clock-gate to 8/8 while the first input
            # DMAs are still streaming in.
            warm = pool_misc.tile([128, 128], bf16, tag="warm")
            nc.vector.memset(warm[:], 0.0)
            with tc.tile_pool(name="warmps", bufs=2, space="PSUM") as pool_wm:
                for i in range(32):
                    ps_w = pool_wm.tile([128, 128], f32, tag="warmps")
                    nc.tensor.matmul(
                        ps_w[:], warm[:], warm[:], start=True, stop=True
                    )

            bqkv_sb = pool_misc.tile([128, 3 * D // 128], f32, tag="bqkv")
            nc.sync.dma_start(bqkv_sb[:], bqkv_pre[:])
            bproj_sb = pool_misc.tile([128, D // 128], f32, tag="bproj")
            nc.sync.dma_start(bproj_sb[:], bproj_pre[:])
            # V-bias rows, partition-broadcast to [128, 512] per fc on GpSimd
            bv_row = pool_misc.tile([1, D], f32, tag="bv_row")
            nc.sync.dma_start(bv_row[:], bv_pre[:])
            bv_bcast = pool_misc.tile([128, D], f32, tag="bv_bcast")
            for fc in range(D // 512):
                nc.gpsimd.partition_broadcast(
                    bv_bcast[:, fc * 512 : (fc + 1) * 512],
                    bv_row[:, fc * 512 : (fc + 1) * 512],
                )

            # ---- load x.T resident: 16 tiles [128, 1024], one per c-tile.
            # fc=0 weight DMAs are interleaved ahead of each xT tile and xT
            # is loaded in halves so the first matmuls start early. ----
            xT_t = []
            w_fc0 = []
            for ct in range(NCT):
                w_t = pool_w512.tile(
                    [128, 512], bf16, name="w_fc0", tag="w512", bufs=24
                )
                nc.sync.dma_start(
                    w_t[:], wv[ct * 128 : (ct + 1) * 128, 0:512]
                )
                w_fc0.append(w_t)
                t_ = pool_xbig.tile(
                    [128, T], bf16, name="xT_t", tag="xbig", bufs=NCT
                )
                for half in range(2):
                    nc.sync.dma_start(
                        t_[:, half * 512 : (half + 1) * 512],
                        xT[
                            ct * 128 : (ct + 1) * 128,
                            half * 512 : (half + 1) * 512,
                        ],
                    )
                xT_t.append(t_)

            # ---- phase 1: V for all heads, token-major [128, 8, 2048] ----
            V_sb = pool_vbig.tile([128, T // 128, D], bf16, tag="vbig")
            with tc.tile_pool(name="p1psum", bufs=8, space="PSUM") as pool_p1:
                # fc = 0: ct-outer so compute starts as soon as each xT
                # tile lands (only xT_t[ct] + w_fc0[ct] gate the chain)
                ps_v = [
                    pool_p1.tile([128, 512], f32, name="vps", tag="vps")
                    for _ in range(T // 128)
                ]
                for ct in range(NCT - 1):
                    for tt in range(T // 128):
                        nc.tensor.matmul(
                            ps_v[tt][:],
                            xT_t[ct][:, tt * 128 : (tt + 1) * 128],
                            w_fc0[ct][:],
                            start=(ct == 0),
                            stop=False,
                        )
                for tt in range(T // 128):
                    # final ct closes bank tt, then its evacuation (DVE)
                    # overlaps the next bank's closing matmul
                    nc.tensor.matmul(
                        ps_v[tt][:],
                        xT_t[NCT - 1][:, tt * 128 : (tt + 1) * 128],
                        w_fc0[NCT - 1][:],
                        start=False,
                        stop=True,
                    )
                    # V := psum + b_v (bias broadcast prestaged on gpsimd)
                    nc.vector.tensor_tensor(
                        V_sb[:, tt, 0:512],
                        ps_v[tt][:],
                        bv_bcast[:, 0:512],
                        AluOpType.add,
                    )
                # fc = 1..3: tt-outer so each PSUM bank's chain closes
                # early and its DVE evacuation overlaps the next chain
                for fc in range(1, D // 512):
                    w_list = []
                    for ct in range(NCT):
                        w_t = pool_w512.tile(
                            [128, 512], bf16, name="w_t", tag="w512", bufs=24
                        )
                        nc.sync.dma_start(
                            w_t[:],
                            wv[
                                ct * 128 : (ct + 1) * 128,
                                fc * 512 : (fc + 1) * 512,
                            ],
                        )
                        w_list.append(w_t)
                    for tt in range(T // 128):
                        ps = pool_p1.tile([128, 512], f32, name="vps", tag="vps")
                        for ct in range(NCT):
                            nc.tensor.matmul(
                                ps[:],
                                xT_t[ct][:, tt * 128 : (tt + 1) * 128],
                                w_list[ct][:],
                                start=(ct == 0),
                                stop=(ct == NCT - 1),
                            )
                        nc.vector.tensor_tensor(
                            V_sb[:, tt, fc * 512 : (fc + 1) * 512],
                            ps[:],
                            bv_bcast[:, fc * 512 : (fc + 1) * 512],
                            AluOpType.add,
                        )

            # ---- phase 2: per-head attention; y.T stays resident ----
            ao_full = [
                pool_aobig.tile([128, T], bf16, name="ao_full", tag="aobig", bufs=H)
                for _ in range(H)
            ]
            with (
                tc.tile_pool(name="sps", bufs=2, space="PSUM") as pool_s,
                tc.tile_pool(name="qaps", bufs=2, space="PSUM") as pool_qa,
                tc.tile_pool(name="yps", bufs=2, space="PSUM") as pool_y,
                tc.tile_pool(name="dps", bufs=2, space="PSUM") as pool_d,
            ):
                for h in range(H):
                    # 2a: Q^T and K^T for head h, d-major [128, 1024]
                    qk = {}
                    for s, (si, btile) in (
                        ("q", (0, h)),
                        ("k", (1, NCT + h)),
                    ):
                        sb = pool_qk.tile([128, T], bf16, tag="qk")
                        w_t = pool_wbig.tile(
                            [128, NCT, 128], bf16, name="w_t", tag="wbig", bufs=5
                        )
                        nc.sync.dma_start(
                            w_t[:],
                            wqk[si, h].rearrange("p (n f) -> p n f", n=NCT),
                        )
                        for jc in range(NQ):
                            ps = pool_qa.tile(
                                [128, 512], f32, name="qkps", tag="qa"
                            )
                            for ct in range(NCT):
                                nc.tensor.matmul(
                                    ps[:],
                                    w_t[:, ct, :],
                                    xT_t[ct][:, jc * 512 : (jc + 1) * 512],
                                    start=(ct == 0),
                                    stop=(ct == NCT - 1),
                                )
                            nc.vector.tensor_scalar_add(
                                sb[:, jc * 512 : (jc + 1) * 512],
                                ps[:],
                                bqkv_sb[:, btile : btile + 1],
                            )
                        qk[s] = sb

                    # 2b: causal attention, scores transposed [k, q].
                    # Diagonal tiles (r >= 0) only touch columns [128r:512];
                    # everything left of that is causally dead. The loop is
                    # software-pipelined one tile deep so scores(ki+1) sits
                    # between scores(ki) and AV(ki) in the in-order PE queue,
                    # hiding the exp+mask latency and the matmul drain.
                    for jc in range(NQ):
                        nk = 4 * jc + 4  # k-tiles 0 .. 4*jc+3 (rest masked)
                        ps_y = pool_y.tile([128, 512], f32, tag="y")
                        ps_d = pool_d.tile([128, 512], f32, tag="d")
                        e_pend = [None] * nk

                        def emit_scores(ki, jc=jc):
                            r = ki - 4 * jc
                            off = 128 * r if r > 0 else 0
                            ps_s = pool_s.tile([128, 512], f32, tag="mm512")
                            nc.tensor.matmul(
                                ps_s[:, off:512],
                                qk["k"][:, ki * 128 : (ki + 1) * 128],
                                qk["q"][:, jc * 512 + off : (jc + 1) * 512],
                                start=True,
                                stop=True,
                            )
                            e_t = pool_e.tile([128, 512], bf16, tag="e")
                            nc.scalar.activation(
                                e_t[:, off:512], ps_s[:, off:512], Exp, scale=SCALE
                            )
                            if r >= 0:
                                # keep where k <= q; in local cols f' = f-off
                                # that is f' >= p (off == 128r cancels)
                                nc.gpsimd.affine_select(
                                    out=e_t[:, off:512],
                                    in_=e_t[:, off:512],
                                    compare_op=mybir.AluOpType.is_ge,
                                    fill=0.0,
                                    base=0,
                                    pattern=[[1, 512 - off]],
                                    channel_multiplier=-1,
                                )
                            return e_t

                        def emit_av(ki, jc=jc, nk=nk, h=h):
                            r = ki - 4 * jc
                            off = 128 * r if r > 0 else 0
                            e_t = e_pend[ki]
                            nc.tensor.matmul(
                                ps_y[:, off:512],
                                V_sb[:, ki, h * 128 : (h + 1) * 128],
                                e_t[:, off:512],
                                start=(ki == 0),
                                stop=(ki == nk - 1),
                            )
                            nc.tensor.matmul(
                                ps_d[:, off:512],
                                ones_mat[:],
                                e_t[:, off:512],
                                start=(ki == 0),
                                stop=(ki == nk - 1),
                            )

                        for ki in range(nk + 1):
                            if ki < nk:
                                e_pend[ki] = emit_scores(ki)
                            if ki >= 1:
                                emit_av(ki - 1)
                        # approx reciprocal (denominators bounded away from 0
                        # by the diagonal exp term); ps_d rows are identical,
                        # so the reciprocal IS already partition-broadcast
                        d_bcast = pool_den.tile(
                            [128, 512], f32, name="d_bcast", tag="bcast"
                        )
                        nc.vector.reciprocal_approx_fast(out=d_bcast[:], in_=ps_d[:])
                        nc.vector.tensor_mul(
                            ao_full[h][:, jc * 512 : (jc + 1) * 512],
                            ps_y[:],
                            d_bcast[:],
                        )

            # ---- phase 3: output projection, emitted transposed ----
            with tc.tile_pool(name="p3psum", bufs=4, space="PSUM") as pool_p3:
                for dt in range(D // 128):
                    wp_t = pool_wproj.tile(
                        [128, NCT, 128], bf16, name="wp_t", tag="wproj", bufs=4
                    )
                    nc.sync.dma_start(
                        wp_t[:],
                        wproj_t[dt].rearrange("p (n f) -> p n f", n=NCT),
                    )
                    for jc in range(NQ):
                        ps = pool_p3.tile([128, 512], f32, tag="mm512")
                        for ct in range(NCT):
                            nc.tensor.matmul(
                                ps[:],
                                wp_t[:, ct, :],
                                ao_full[ct][:, jc * 512 : (jc + 1) * 512],
                                start=(ct == 0),
                                stop=(ct == NCT - 1),
                            )
                        o_t = pool_out.tile([128, 512], f32, tag="outp")
                        nc.vector.tensor_scalar_add(
                            o_t[:], ps[:], bproj_sb[:, dt : dt + 1]
                        )
                        nc.sync.dma_start(
                            outT[dt * 128 : (dt + 1) * 128, jc * 512 : (jc + 1) * 512],
                            o_t[:],
                        )

    nc.compile()
    return nc


def _get_nc():
    if "nc" not in _CACHE:
        _CACHE["nc"] = _build()
    return _CACHE["nc"]


def kernel(x, w_qkv, b_qkv, w_proj, b_proj, _trace=False, _trace_kwargs=None):
    import ml_dtypes
    from concourse.bass_utils import run_bass_kernel_spmd

    bf16 = ml_dtypes.bfloat16
    x = np.asarray(x, dtype=np.float32)
    w_qkv = np.asarray(w_qkv, dtype=np.float32)
    b_qkv = np.asarray(b_qkv, dtype=np.float32)
    w_proj = np.asarray(w_proj, dtype=np.float32)
    b_proj = np.asarray(b_proj, dtype=np.float32)

    # biases pre-arranged to [128, n] feature-tile columns (contiguous
    # partition lines -> cheap DMAs)
    bqkv_pre = np.ascontiguousarray(b_qkv.reshape(3 * D // 128, 128).T)
    bproj_pre = np.ascontiguousarray(b_proj.reshape(D // 128, 128).T)
    bv_pre = np.ascontiguousarray(b_qkv[2 * D : 3 * D].reshape(1, D))

    # host-side weight relayouts (one DMA with contiguous 4KB partition
    # lines per Q/K head block and per out-proj block)
    wv = np.ascontiguousarray(w_qkv[:, 2 * D : 3 * D]).astype(bf16)
    # [s, h, p, ct*128+f] = w_qkv[ct*128+p, s*D + h*128 + f]
    wqk_f = (
        w_qkv[:, : 2 * D]
        .reshape(NCT, 128, 2, H, DH)
        .transpose(2, 3, 1, 0, 4)
        .reshape(2, H, 128, D)
    )
    wqk = np.ascontiguousarray(wqk_f).astype(bf16)
    # [dt, p, ct*128+f] = w_proj[ct*128+p, dt*128+f]
    wproj_f = (
        w_proj.reshape(NCT, 128, D // 128, 128)
        .transpose(2, 1, 0, 3)
        .reshape(D // 128, 128, D)
    )
    wproj_t = np.ascontiguousarray(wproj_f).astype(bf16)

    nc = _get_nc()
    in_maps = []
    for i in range(N_CORES):
        in_maps.append(
            {
                "xT": np.ascontiguousarray(x[i].T).astype(bf16),
                "wv": wv,
                "wqk": wqk,
                "wproj_t": wproj_t,
                "bqkv_pre": bqkv_pre,
                "bproj_pre": bproj_pre,
                "bv_pre": bv_pre,
            }
        )
    res = run_bass_kernel_spmd(
        nc,
        in_maps,
        list(range(N_CORES)),
        trace=_trace,
        **(_trace_kwargs or {}),
    )
    y = np.stack(
        [np.ascontiguousarray(res.results[i]["outT"].T) for i in range(N_CORES)]
    )
    if _trace:
        _CACHE["last_result"] = res
    return y
